# revision 52
# baseline (speedup 1.0000x reference)
"""Causal self-attention (B=4, T=1024, C=768, 12 heads) on 8 trn2 cores.

Sharding: core c = (batch b=c//2, head-group hg=c%2 of 6 heads).
Each core: QKV projection for its head-group (TP column split of Wqkv),
causal attention for 6 heads, partial output projection (TP row split of
Wproj). Host sums the two partials per batch (the all-reduce) and
transposes back.

Device-side layouts (contraction dim always on partitions):
  x^T  [C=768, T=1024]  DMA'd in 8 column slabs (host pre-slabs the dram
       layout) so the first matmuls can start ~4us earlier
  q^T/k^T = W^T x^T as [cols, T]  (lhsT=Wqk slice, rhs=x^T)
  v = x W_v as [T, cols]          (lhsT=x^T chunk, rhs=Wv)
  scoresT [T_k, T_q] = k_h q_h^T  (lhsT=k_h^T, rhs=q_h^T)
  softmax without max-subtraction (scores ~ N(0, 0.1); exp is safe),
  denominator via a ones-column appended to v (row 64 of att@[v|1]),
  out_h^T [64, T_q] = [v|1]^T attT, proj^T = Wp_hg^T out^T.

Schedule: the PE stream (~73us busy) is the binding resource; the ACT
exp stream (~34us) and DVE norm chain are arranged to overlap it.
 - qk/proj biases are fused into the PSUM->SBUF drains (per-partition
   bias on ACT add / DVE scalar_tensor_tensor); the v bias is a K=1
   ones-row matmul (its bias varies along the free dim).
 - The causal mask is a 0/1 bf16 multiply on the 128-wide diagonal band
   of att2 (DVE), not identity-matmul accumulation on the PE.
 - Attention (qb-major pair order) is interleaved with the remaining
   qk projection / v / proj matmuls as "filler" units, so the PE
   streams continuously while ACT works through exp; proj tb=0 fills
   the qb=1 pairs once all qb=0 norms exist.
 - GpSimd runs ONLY partition_broadcast: mixing its custom-lib op with
   standard tensor ops forces ~5.6us microcode library reloads.
 - PSUM: score ring 2x2 banks, filler ring 2x1, AV accumulator 1x2.
NOTE: custom DVE ops (reciprocal_approx_fast) require base_partition 0
inputs on HW.
"""

import numpy as np
import ml_dtypes

B, T, C = 4, 1024, 768
NH, HD = 12, 64
HPC = NH // 2          # heads per core = 6
QKCOLS = 2 * HPC * HD  # 768 (q then k cols for this head group)
VC = HPC * HD          # 384
NCORES = 8
TB = 512               # matmul moving free-dim block
BF16 = ml_dtypes.bfloat16

_prog = None


def _build_program():
    import concourse.bass as bass
    import concourse.tile as tile
    from concourse import bacc, mybir

    f32 = mybir.dt.float32
    bf16 = mybir.dt.bfloat16

    nc = bacc.Bacc(
        "TRN2", target_bir_lowering=False, debug=False, enable_asserts=False
    )

    xTs = nc.dram_tensor("xTs", [8, 128, 6 * 128], bf16, kind="ExternalInput")
    wqks = nc.dram_tensor("wqks", [6, 128, 6 * 128], bf16, kind="ExternalInput")
    wv = nc.dram_tensor("wv", [C, VC], bf16, kind="ExternalInput")
    wp = nc.dram_tensor("wp", [VC, C], bf16, kind="ExternalInput")
    fcb = nc.dram_tensor("fcb", [128, 12], f32, kind="ExternalInput")
    frow = nc.dram_tensor("frow", [1, VC], bf16, kind="ExternalInput")
    maskd = nc.dram_tensor("maskd", [128, 2 * 128], bf16, kind="ExternalInput")
    out = nc.dram_tensor("out", [C, T], bf16, kind="ExternalOutput")

    Exp = mybir.ActivationFunctionType.Exp
    ADD = mybir.AluOpType.add
    MULT = mybir.AluOpType.mult

    with tile.TileContext(nc) as tc:
        with (
            tc.tile_pool(name="consts", bufs=1) as consts,
            tc.tile_pool(name="psA", bufs=2, space="PSUM") as psA,
            tc.tile_pool(name="psB", bufs=2, space="PSUM") as psB,
            tc.tile_pool(name="psC", bufs=1, space="PSUM") as psC,
            tc.tile_pool(name="work", bufs=1) as work,
        ):
            # slab-major: [p, slab tk, kc*128+tl] — slab DMAs land contiguous
            # 1536B-per-partition lines (kc-major layout fragments the dst
            # into 256B segments and halves DMA throughput)
            xT_sb = consts.tile([128, 8, 6 * 128], bf16)
            wqk_sb = consts.tile([128, 6, 6 * 128], bf16)   # [p, cb, kc*128+col]
            wv_sb = consts.tile([128, 6, VC], bf16)
            wp_sb = consts.tile([128, 3, C], bf16)
            qk_sb = consts.tile([128, 6, T], bf16)   # q^T (blocks 0-2), k^T (3-5)
            v_sb = consts.tile([128, 8, HPC, HD + 1], bf16)
            out_sb = consts.tile([128, 3, T], bf16)
            fcb_sb = consts.tile([128, 12], f32)     # bp (0-5) | bqk (6-11) per-partition
            frow_sb = consts.tile([1, VC], bf16)           # bv row
            mask_sb = consts.tile([128, 2, 128], bf16)     # 0/1 tril band, both halves
            ones_sb = consts.tile([1, TB], bf16)
            ones128 = consts.tile([128, TB], bf16)
            wz = consts.tile([128, TB], bf16)

            # ---- memsets first: the warmup and the K=1 bias matmuls need
            # these, and the issuing engines are about to be busy with DMAs
            nc.vector.memset(v_sb[:, :, :, HD : HD + 1], 1.0)
            nc.vector.memset(wz[:], 0.0)
            nc.gpsimd.memset(ones_sb[:], 1.0)
            nc.gpsimd.memset(ones128[:], 1.0)

            # ---- input DMAs, priority order round-robined over 3 queues ----
            eng = [nc.sync, nc.gpsimd, nc.scalar]
            xTs_r = xTs.rearrange("s p c -> p s c")
            wqks_r = wqks.rearrange("b p c -> p b c")
            wv_r = wv.rearrange("(a p) c -> p a c", p=128)
            wp_r = wp.rearrange("(a p) c -> p a c", p=128)
            dmas = []

            def slab(tk):
                dmas.append((xT_sb[:, tk, :], xTs_r[:, tk, :]))

            def wqkcb(cb):
                dmas.append((wqk_sb[:, cb, :], wqks_r[:, cb, :]))

            dmas.append((frow_sb[:], frow[:]))
            dmas.append((fcb_sb[:], fcb[:]))
            dmas.append((mask_sb[:].rearrange("p a t -> p (a t)"), maskd[:]))
            for tk in range(4):
                slab(tk)
            wqkcb(0)
            wqkcb(3)
            dmas.append((wv_sb[:, 0:3, :], wv_r[:, 0:3, :]))
            dmas.append((wv_sb[:, 3:6, :], wv_r[:, 3:6, :]))
            wqkcb(1)
            wqkcb(4)
            wqkcb(2)
            wqkcb(5)
            for tk in range(4, 8):
                slab(tk)
            dmas.append((wp_sb[:], wp_r[:]))
            for i, (dst, src) in enumerate(dmas):
                eng[i % 3].dma_start(dst, src)

            # ---- HAM warm-up: cover the whole input-DMA window so the PE
            # never idles >3.4us early and the real matmuls start at 2.4GHz
            for w in range(12):
                ps_w = psA.tile([128, 2, TB], f32, tag="att", name="ps_w")
                nc.tensor.matmul(
                    ps_w[:, 0, :], wz[:, 0:128], wz[:], start=True, stop=True
                )

            # ---- filler units (qk-proj groups, v tiles, proj blocks) ----
            # each unit is ~0.3-0.5us of PE work + a PSUM drain. GpSimd
            # cannot touch PSUM, so PSUM drains go to DVE/ACT; GpSimd gets
            # the SBUF-only work (broadcasts, norm multiplies).

            def qk_units(cb, tb):
                """3 units for one (cb, tb) group in a 1-bank psB tile.
                The bias is fused into the PSUM->SBUF drain (per-partition
                bias AP), so the group is just the 6 contraction matmuls."""
                t = psB.tile([128, TB], f32, tag="fill", name="ps_qk")

                def u(part, t=t, cb=cb, tb=tb):
                    def run():
                        kcs = ((0, 1), (2, 3), (4, 5))[part]
                        for kc in kcs:
                            nc.tensor.matmul(
                                t[:],
                                wqk_sb[:, cb, kc * 128 : (kc + 1) * 128],
                                xT_sb[:, tb * 4 : tb * 4 + 4,
                                      kc * 128 : (kc + 1) * 128],
                                start=(kc == 0),
                                stop=(kc == 5),
                            )
                        if part == 2:
                            # cb0/cb3 drain pre-attention on the idle DVE;
                            # later groups drain on ACT (DVE is norm-loaded)
                            ts = slice(tb * TB, (tb + 1) * TB)
                            dst = qk_sb[:, cb, ts]
                            if cb in (0, 3):
                                nc.vector.scalar_tensor_tensor(
                                    dst, t[:], fcb_sb[:, 6 + cb : 7 + cb],
                                    ones128[:], ADD, MULT,
                                )
                            else:
                                nc.scalar.add(
                                    dst, t[:], fcb_sb[:, 6 + cb : 7 + cb]
                                )

                    return run

                return [u(0), u(1), u(2)]

            def v_units(tk):
                """2 units for one v tile in a 1-bank psB tile."""
                t = psB.tile([128, TB], f32, tag="fill", name="ps_v")

                def u(part, t=t, tk=tk):
                    def run():
                        ph = t[:, 0:VC]
                        if part == 0:
                            nc.tensor.matmul(
                                ph,
                                ones_sb[0:1, 0:128],
                                frow_sb[0:1, :],
                                start=True,
                                stop=False,
                            )
                            kcs = (0, 1, 2)
                        else:
                            kcs = (3, 4, 5)
                        for kc in kcs:
                            nc.tensor.matmul(
                                ph,
                                xT_sb[:, tk, kc * 128 : (kc + 1) * 128],
                                wv_sb[:, kc, :],
                                start=False,
                                stop=(kc == 5),
                            )
                        if part == 1:
                            nc.vector.tensor_copy(
                                v_sb[:, tk, :, 0:HD],
                                ph.rearrange("p (h d) -> p h d", h=HPC),
                            )

                    return run

                return [u(0), u(1)]

            def proj_units(tb):
                """6 units (one per ob), each a 1-bank psB tile."""
                units = []
                for ob in range(6):
                    t = psB.tile([128, TB], f32, tag="fill", name="ps_pr")

                    def run(ob=ob, t=t, tb=tb):
                        for r in range(3):
                            nc.tensor.matmul(
                                t[:],
                                wp_sb[:, r, ob * 128 : (ob + 1) * 128],
                                out_sb[:, r, tb * TB : (tb + 1) * TB],
                                start=(r == 0),
                                stop=(r == 2),
                            )
                        res = work.tile([128, TB], bf16, tag="res", bufs=3)
                        if ob % 2 == 0 or tb == 0:
                            nc.scalar.add(res[:], t[:], fcb_sb[:, ob : ob + 1])
                        else:
                            nc.vector.scalar_tensor_tensor(
                                res[:], t[:], fcb_sb[:, ob : ob + 1],
                                ones128[:], ADD, MULT,
                            )
                        eng[ob % 3].dma_start(
                            out[ob * 128 : (ob + 1) * 128,
                                tb * TB : (tb + 1) * TB],
                            res[:],
                        )

                    units.append(run)
                return units

            # ---- pre-attention PE work: just enough q/k for pair 0 ----
            for u in qk_units(0, 0) + qk_units(3, 0):
                u()

            fillers = []
            marks = {}
            fillers += v_units(0) + v_units(1)
            fillers += v_units(2) + v_units(3)
            fillers += qk_units(1, 0) + qk_units(4, 0)
            marks[(1, 0)] = len(fillers)
            fillers += qk_units(2, 0) + qk_units(5, 0)
            marks[(2, 0)] = len(fillers)
            fillers += qk_units(0, 1) + qk_units(3, 1)
            marks[(0, 1)] = len(fillers)     # q cb0-tb1 / k cb3-tb1 ready
            fillers += v_units(4) + v_units(5)
            fillers += v_units(6) + v_units(7)
            fillers += qk_units(1, 1) + qk_units(4, 1)
            marks[(1, 1)] = len(fillers)
            fillers += qk_units(2, 1) + qk_units(5, 1)
            marks[(2, 1)] = len(fillers)

            consumed = [0]

            def pump(n):
                for _ in range(n):
                    if fillers:
                        fillers.pop(0)()
                        consumed[0] += 1

            def pump_until(mark):
                while consumed[0] < mark and fillers:
                    pump(1)

            # just enough pre-attention fill that (1,0)'s inputs (mark 14)
            # are emitted by its start; the rest rides the qb=0 phases'
            # pump-2 slots below — no phase-start bursts, exp starts ~19us
            pump(6)

            # ---- attention, interleaved; qb-major so the proj tb=0 work
            # (gated on all three qb=0 norms) spreads over the qb=1 pairs ----
            pend = []  # deferred AV / normalization tasks
            for qb in range(2):
                for j in range(3):
                    qblk, kblk = j, 3 + j
                    hA, hB = 2 * j, 2 * j + 1
                    nkb = 4 * (qb + 1)
                    oe2 = psC.tile([65, 2, TB], f32, tag="acc", name="oe2")

                    if j == 2 and qb == 1:
                        # proj tb=0 (gated only on the long-done qb=0
                        # norms) is the last pair's filler supply
                        fillers += proj_units(0)
                    if (j, qb) in marks:
                        # emit every filler this (j, qb)'s QKs depend on
                        # before those QKs are emitted
                        pump_until(marks[(j, qb)])

                    def qk_exp(kb, qblk=qblk, kblk=kblk, qb=qb):
                        stair = kb >= qb * 4
                        o = (kb - qb * 4) * 128 if stair else 0
                        qs = slice(qb * TB + o, (qb + 1) * TB)
                        ks = slice(kb * 128, (kb + 1) * 128)
                        ps2 = psA.tile([128, 2, TB], f32, tag="att", name="ps2")
                        nc.tensor.matmul(
                            ps2[:, 0, o:], qk_sb[0:64, kblk, ks],
                            qk_sb[0:64, qblk, qs], start=True, stop=True,
                        )
                        nc.tensor.matmul(
                            ps2[:, 1, o:], qk_sb[64:128, kblk, ks],
                            qk_sb[64:128, qblk, qs], start=True, stop=True,
                        )
                        att2 = work.tile([128, 2, TB], bf16, tag="att2", bufs=10)
                        # exp(score/8); softmax max-subtraction skipped
                        nc.scalar.activation(
                            att2[:, :, o:], ps2[:, :, o:], Exp, scale=0.125
                        )
                        if stair:
                            # zero the dead upper-triangle of the 128-wide
                            # diagonal band (columns beyond o+128 are all live)
                            nc.vector.tensor_mul(
                                att2[:, :, o : o + 128],
                                att2[:, :, o : o + 128],
                                mask_sb[:],
                            )
                        return o, att2

                    def av(kb, o, att2, oe2=oe2, hA=hA, hB=hB, nkb=nkb):
                        for i, h in ((0, hA), (1, hB)):
                            nc.tensor.matmul(
                                oe2[:, i, o:],
                                v_sb[:, kb, h, :],
                                att2[:, i, o:],
                                start=(kb == 0),
                                stop=(kb == nkb - 1),
                            )

                    for kb in range(nkb):
                        item = (kb, *qk_exp(kb))
                        pend.append(lambda it=item, fn=av: fn(*it))
                        if j == 2 and qb == 1:
                            pump(1)   # supply here is the 6 proj tb=0 units
                        else:
                            pump(2 if (qb == 0 or kb < 4) else 1)
                        while len(pend) > 2:
                            pend.pop(0)()

                    def norm(oe2=oe2, qblk=qblk, qb=qb):
                        # per-head pipelined chain: den -> recip(head) ->
                        # broadcast(head) -> mul(head); oecp (which only
                        # gates the PSUM accumulator's reuse) runs in the
                        # broadcast's shadow, not ahead of the chain
                        den2 = work.tile([1, 2 * TB], f32, tag="den", bufs=2)
                        nc.vector.tensor_copy(
                            den2[:].rearrange("p (a f) -> p a f", a=2),
                            oe2[64:65, :, :],
                        )
                        rden2 = work.tile([1, 2 * TB], f32, tag="rden", bufs=2)
                        rdb2 = work.tile([64, 2 * TB], f32, tag="rdb", bufs=2)
                        nc.vector.reciprocal_approx_fast(
                            rden2[:, 0:TB], den2[:, 0:TB]
                        )
                        nc.gpsimd.partition_broadcast(
                            rdb2[:, 0:TB], rden2[:, 0:TB]
                        )
                        nc.vector.reciprocal_approx_fast(
                            rden2[:, TB:], den2[:, TB:]
                        )
                        oecp = work.tile([64, 2, TB], bf16, tag="oecp", bufs=2)
                        nc.vector.tensor_copy(oecp[:], oe2[0:64, :, :])
                        nc.gpsimd.partition_broadcast(
                            rdb2[:, TB:], rden2[:, TB:]
                        )
                        for i in range(2):
                            nc.vector.tensor_mul(
                                out_sb[
                                    i * 64 : (i + 1) * 64,
                                    qblk,
                                    qb * TB : (qb + 1) * TB,
                                ],
                                oecp[:, i, :],
                                rdb2[:, i * TB : (i + 1) * TB],
                            )

                    pend.append(norm)
                    if j == 2:
                        # eager drain: j2-qb0 norm gates proj tb=0; j2-qb1
                        # norm gates the proj tb=1 tail
                        while pend:
                            pend.pop(0)()

            pump(len(fillers))
            # keep-warm dummies: while proj tb=1 waits for the final norm
            # chain the PE would otherwise idle >3.4us and HAM re-throttles
            # the whole proj tail to 1.2 GHz
            for w in range(20):
                ps_w = psA.tile([128, 2, TB], f32, tag="att", name="ps_w2")
                nc.tensor.matmul(
                    ps_w[:, 0, :], wz[:, 0:128], wz[:], start=True, stop=True
                )
            for u in proj_units(1):
                u()

    nc.compile()
    return nc


def _get_prog():
    global _prog
    if _prog is None:
        _prog = _build_program()
    return _prog


def make_in_maps(x, Wqkv, bqkv, Wproj, bproj):
    """Host-side sharding: per-core input dict."""
    x = np.asarray(x, dtype=np.float32)
    Wqkv = np.asarray(Wqkv, dtype=np.float32)
    bqkv = np.asarray(bqkv, dtype=np.float32)
    Wproj = np.asarray(Wproj, dtype=np.float32)
    bproj = np.asarray(bproj, dtype=np.float32)

    # 0/1 lower-triangle band mask [128, 128], duplicated for both heads
    f = np.arange(128)[None, :]
    p = np.arange(128)[:, None]
    band = (f >= p).astype(np.float32)
    maskd = np.concatenate([band, band], axis=1).astype(BF16)

    in_maps = []
    for c in range(NCORES):
        b, hg = c // 2, c % 2
        qcols = slice(hg * VC, (hg + 1) * VC)
        kcols = slice(C + hg * VC, C + (hg + 1) * VC)
        vcols = slice(2 * C + hg * VC, 2 * C + (hg + 1) * VC)
        wqk_c = np.concatenate([Wqkv[:, qcols], Wqkv[:, kcols]], axis=1)
        bqk_c = np.concatenate([bqkv[qcols], bqkv[kcols]])
        bp_c = bproj if hg == 0 else np.zeros_like(bproj)

        xT = np.ascontiguousarray(x[b].T)                     # [C, T]
        # slab-major: [tk, p, kc*128+tl]
        xTs = (
            xT.reshape(6, 128, 8, 128).transpose(2, 1, 0, 3).reshape(8, 128, 768)
        )
        # cb-major: [cb, p, kc*128+col]
        wqks = (
            wqk_c.reshape(6, 128, 6, 128)        # [kc, p, cb, col]
            .transpose(2, 1, 0, 3)               # [cb, p, kc, col]
            .reshape(6, 128, 768)
        )

        in_maps.append(
            {
                "xTs": np.ascontiguousarray(xTs).astype(BF16),
                "wqks": np.ascontiguousarray(wqks).astype(BF16),
                "wv": np.ascontiguousarray(Wqkv[:, vcols]).astype(BF16),
                "wp": np.ascontiguousarray(
                    Wproj[hg * VC : (hg + 1) * VC, :]
                ).astype(BF16),
                "fcb": np.ascontiguousarray(
                    np.concatenate(
                        [bp_c.reshape(6, 128).T, bqk_c.reshape(6, 128).T],
                        axis=1,
                    )
                ).astype(np.float32),
                "frow": bqkv[vcols][None, :].astype(BF16),
                "maskd": maskd,
            }
        )
    return in_maps


def gather_output(results):
    """results: per-core dict with 'out' [768, 1024] partials."""
    outs = []
    for b in range(B):
        part = results[2 * b]["out"].astype(np.float32) + results[2 * b + 1][
            "out"
        ].astype(np.float32)
        outs.append(part.T)
    return np.stack(outs).astype(np.float32)


def run(inputs, trace=False):
    from concourse.bass_utils import run_bass_kernel_spmd

    nc = _get_prog()
    in_maps = make_in_maps(
        inputs["x"], inputs["Wqkv"], inputs["bqkv"], inputs["Wproj"], inputs["bproj"]
    )
    res = run_bass_kernel_spmd(nc, in_maps, list(range(NCORES)), trace=trace)
    return gather_output(res.results), res


def kernel(**inputs):
    out, _ = run(inputs, trace=False)
    return out
